# revision 57
# baseline (speedup 1.0000x reference)
"""Trainium2 Bass kernel for nn_DigitConvolutionalModel (dense_cnn).

Model: y = relu(conv3x3(x) @ w1.T + b1) @ w2.T + b2, x: [65536, 784] f32.

Strategy:
  * The 3x3 valid conv (784 -> 676) and FC1 (676 -> 128) are both linear,
    so they fuse on the host into one effective weight W1e = w1 @ C with
    shape [128, 784] (C is the sparse conv operator). The device then runs
    a pure GEMM pipeline: y = relu(x @ W1e.T + b1) @ w2.T + b2.
  * Pure data parallel over 8 NeuronCores: each core gets 8192 rows of x.
    No collectives; each core produces its own output shard.
  * Matmul operands travel as fp16 (see MM_MODE below): tf32-class
    accuracy for this model's value ranges, 1 cycle/row on the PE, and
    half the HBM traffic for x. All accumulation stays fp32 in PSUM.
  * Per-core x shards are pre-tiled on the host into the exact device
    layout [load, partition, chunk, col] so every 1.6 MB x DMA is one
    fully contiguous region (128 descriptors x 12 KB) — the contraction
    dim lands on SBUF partitions with no on-chip transposes (DMA x-bar
    transpose wouldn't help: 2-byte dtype only, and strided fallbacks are
    descriptor-bound). The K=784 contraction is split 6x128 + 16 so the
    bulk loads use all 128 partitions / 16 SDMA engines; the 16-row tail
    for the whole batch is one upfront 256 KB load.
  * Per 1024-column load: two PSUM banks of 7 accumulating FC1 matmuls
    each (si-outer so bank 0's relu overlaps bank 1's matmuls), fused
    bias+ReLU on the vector engine (PSUM -> SBUF fp16), one [10, 512]
    FC2 matmul per bank, FC2 bias on the scalar engine, SWDGE store.
    Output returns as yT [10, 8192] per core; the host transposes/concats.
  * Tile/walrus quirks handled explicitly: this walrus allows ONE sync
    wait per instruction, so multi-waits are split into event-semaphore
    chains (bass_rust.generate_event_semaphores) and tiny dummy bf16
    ldweights "probes" absorb cross-engine waits into the PE stream ahead
    of each matmul group. A dozen dummy matmuls during the DMA-bound
    startup window pre-warm the PE's HAM clock gate to 2.4 GHz.
"""

import os

import numpy as np

import concourse.bass as bass
import concourse.mybir as mybir
import concourse.tile as tile
from concourse.bass import ts
from concourse.bass_utils import run_bass_kernel_spmd

H = W = 28
KH = KW = 3
CIN = H * W  # 784
HID = 128
OUT = 10
B_TOTAL = 65536
NCORES = 8
BS = B_TOTAL // NCORES  # 8192 rows per core
NB = 512  # batch columns per psum block (fp32 PSUM bank limit)
NBLK = BS // NB  # 16
NLOAD = 1024  # batch columns per x DMA (~3.2 MB transfers)
NSUB = NLOAD // NB  # psum blocks per load
# contraction split: 6 full-partition chunks of 128 (keeps all 16 SDMA
# engines loaded on the big x DMAs) + a 16-row tail chunk
KCH = 128
KC = 6  # full chunks (6 * 128 = 768)
KTAIL = CIN - KC * KCH  # 16

# Matmul operand dtype. fp16 (e5m10): tf32-class accuracy for this model's
# value ranges (|x|<6, |h|<13), 1 cycle/row on the PE with fast weight
# load, and half the HBM bytes for x. "f32r" = single-pass reduced fp32
# (same accuracy class, but 4-byte DMA traffic); "f32" = exact.
MM_MODE = os.environ.get("BASS_MM_DT", "f16")
if os.environ.get("BASS_FP32R") == "0":  # legacy switch used by simcheck
    MM_MODE = "f32"
HOST_DT = np.float16 if MM_MODE == "f16" else np.float32


def _build_nc():
    f32 = mybir.dt.float32
    # Matmul-operand dtype. (For f32r, the BIR verifier requires fp32r
    # matmul inputs to be *typed* fp32r at their producer, so the DRAM
    # tensors and SBUF tiles feeding matmuls carry this dtype.)
    mdt = {
        "f16": mybir.dt.float16,
        "f32r": mybir.dt.float32r,
        "f32": f32,
    }[MM_MODE]
    nc = bass.Bass()
    # x big part, host-pretiled to [load, partition, chunk, col] so each
    # per-load DMA is one contiguous region (128 descriptors x 24 KB)
    xb = nc.dram_tensor(
        "xb", [BS // NLOAD, KCH, KC, NLOAD], mdt, kind="ExternalInput"
    )
    # x contraction tail (features 768..783) for the whole batch
    xtl = nc.dram_tensor("xtl", [KTAIL, BS], mdt, kind="ExternalInput")
    w1t = nc.dram_tensor("w1t", [CIN, HID], mdt, kind="ExternalInput")
    b1d = nc.dram_tensor("b1d", [HID, 1], f32, kind="ExternalInput")
    w2t = nc.dram_tensor("w2t", [HID, OUT], mdt, kind="ExternalInput")
    b2d = nc.dram_tensor("b2d", [OUT, 1], f32, kind="ExternalInput")
    yt = nc.dram_tensor("yt", [OUT, BS], f32, kind="ExternalOutput")

    with tile.TileContext(nc) as tc:
        with (
            tc.tile_pool(name="consts", bufs=1) as consts,
            # buffer depths sized for the 2-byte path; the 4-byte fallback
            # modes halve them to stay inside SBUF
            tc.tile_pool(name="xin", bufs=6 if MM_MODE == "f16" else 3) as xin,
            tc.tile_pool(name="hpool", bufs=16 if MM_MODE == "f16" else 8) as hpool,
            tc.tile_pool(name="opool", bufs=8) as opool,
            tc.tile_pool(name="ps1", bufs=4, space="PSUM") as ps1p,
            tc.tile_pool(name="ps2", bufs=2, space="PSUM") as ps2p,
        ):
            # Tapered block schedule: 512-column blocks at the start (so the
            # first FC1 only waits on a quarter-size transfer) and at the
            # end (shorter pipeline drain); full 1024-column blocks between.
            sched = (
                [(0, NB), (NB, NB)]
                + [(cs, NLOAD) for cs in range(NLOAD, BS - NLOAD, NLOAD)]
                + [(BS - NLOAD, NB), (BS - NB, NB)]
            )

            def x_src(colstart, ncols):
                li, off = divmod(colstart, NLOAD)
                return xb[li][:, :, off : off + ncols]

            # Issue the first x block before anything else so the main DMA
            # stream starts as early as possible (weights are tiny and only
            # gate the PE, which has plenty of slack).
            x_first = xin.tile([KCH, KC, sched[0][1]], mdt, tag="x_s")
            nc.sync.dma_start(x_first[:], x_src(*sched[0]))

            # FC1 weight, chunked [k, chunk, hid]: partition k in 0..127,
            # chunk c selects rows c*128..c*128+127 of w1t; plus 16-row tail.
            w1_t = consts.tile([KCH, KC, HID], mdt)
            nc.sync.dma_start(
                w1_t[:], w1t[0 : KC * KCH, :].rearrange("(c k) m -> k c m", k=KCH)
            )
            w1_tail = consts.tile([KTAIL, HID], mdt)
            nc.sync.dma_start(w1_tail[:], w1t[KC * KCH :, :])
            b1_t = consts.tile([HID, 1], f32)
            nc.sync.dma_start(b1_t[:], b1d[:])
            w2_t = consts.tile([HID, OUT], mdt)
            nc.sync.dma_start(w2_t[:], w2t[:])
            b2_t = consts.tile([OUT, 1], f32)
            nc.sync.dma_start(b2_t[:], b2d[:])

            # Pre-touch the bias tiles on their consumer engines (b1 on DVE,
            # b2 on ACT) so the relu / bias-add instructions don't need a
            # second sync-wait for the bias DMA (walrus: 1 wait per inst).
            b1_probe = consts.tile([1, 1], f32)
            nc.vector.tensor_copy(b1_probe[:], b1_t[0:1, 0:1])
            b2_probe = consts.tile([1, 1], f32)
            nc.scalar.copy(b2_probe[:], b2_t[0:1, 0:1])

            # Matmuls self-load their weights, so every semaphore wait lands
            # on the Matmult itself — and walrus only allows one sync-wait
            # there. Tiny dummy bf16 ldweights "probes" reading 1 element of
            # a tile absorb the cross-engine waits into the PE's in-order
            # stream before each matmul group. The loaded garbage weight is
            # irrelevant (the real matmuls self-load).
            def probe(ap):
                nc.tensor.ldweights(ap[0:1, 0:1].bitcast(mybir.dt.bfloat16))

            # the 16-row contraction tail for the whole batch, loaded once
            x_tl = consts.tile([KTAIL, BS], mdt)
            nc.sync.dma_start(x_tl[:], xtl[:])

            probe(w1_t[:, 0, :])
            probe(w1_tail[:])
            probe(x_tl[:])
            probe(w2_t[:])

            # HAM warm-up: the PE clock gate defaults to 1.2 GHz and only
            # ramps to 2.4 GHz after ~3.4us of sustained activity. The PE is
            # idle anyway until the first x load lands (~13us), so burn that
            # window on dummy matmuls over a zeroed scratch tile; the real
            # matmuls then start at full clock.
            scratch = consts.tile([KCH, NB], mdt)
            nc.gpsimd.memset(scratch[:], 0.0)
            psd = ps1p.tile([HID, NB], f32, tag="ps")
            for _ in range(4):
                nc.tensor.matmul(
                    psd[:], scratch[:, 0:HID], scratch[:], start=True, stop=True
                )

            # Per block: FC1 runs si-outer (bank si finishes early so its
            # relu overlaps the other bank's matmuls), then per-bank
            # relu -> FC2, then one bias-add + store for the whole block.
            for bi, (colstart, ncols) in enumerate(sched):
                nsub = ncols // NB
                if bi == 0:
                    x_t = x_first
                else:
                    tag = "x_t" if ncols == NLOAD else "x_s"
                    x_t = xin.tile([KCH, KC, ncols], mdt, tag=tag)
                    nc.sync.dma_start(x_t[:], x_src(colstart, ncols))

                probe(x_t[:, 0, :])
                # FC1 alternates psum banks between consecutive matmuls so
                # one matmul's PSUM drain overlaps the next one's fill
                # (same-bank accumulation serializes them).
                pss = []
                for si in range(nsub):
                    ps_si = ps1p.tile([HID, NB], f32, tag="ps")
                    pss.append(ps_si)
                for c in range(KC):
                    for si in range(nsub):
                        nc.tensor.matmul(
                            pss[si][:],
                            w1_t[:, c, :],
                            x_t[:, c, ts(si, NB)],
                            start=(c == 0),
                            stop=False,
                        )
                for si in range(nsub):
                    nc.tensor.matmul(
                        pss[si][:],
                        w1_tail[:],
                        x_tl[:, ts(colstart // NB + si, NB)],
                        start=False,
                        stop=True,
                    )

                # ps2 is always the 2-bank shape so all blocks share slots
                ps2 = ps2p.tile([OUT, NSUB, NB], f32, tag="ps2")
                for si in range(nsub):
                    # relu+bias on DVE: h = max(ps + b1, 0)
                    h = hpool.tile([HID, NB], mdt, tag="h")
                    nc.vector.tensor_scalar(
                        h[:],
                        pss[si][:],
                        b1_t[:],
                        0.0,
                        mybir.AluOpType.add,
                        mybir.AluOpType.max,
                    )
                    probe(h[:])
                    nc.tensor.matmul(
                        ps2[:, si, :], w2_t[:], h[:], start=True, stop=True
                    )

                # FC2 bias on the (otherwise idle) scalar engine, store via
                # SWDGE so the ACT sequencer isn't serialized behind it
                o = opool.tile([OUT, nsub, NB], f32, tag="o")
                nc.scalar.activation(
                    o[:],
                    ps2[:, 0:nsub, :],
                    mybir.ActivationFunctionType.Identity,
                    bias=b2_t[:],
                )
                nc.gpsimd.dma_start(yt[:, colstart : colstart + ncols], o[:])

    # This walrus build allows one sync-wait per instruction; Tile emits
    # multi-waits (e.g. slot-recycle WAW + readers-release on DMAs). Split
    # them into event-semaphore chains, same as bacc.compile() does.
    import bass_rust

    bass_rust.generate_event_semaphores(nc)
    return nc


def _fuse_conv_fc1(conv_w, w1):
    """W1e = w1 @ C where C is the 3x3 valid-conv operator [676, 784]."""
    cw = np.asarray(conv_w, np.float64).reshape(KH, KW)
    w1_r = np.asarray(w1, np.float64).reshape(HID, H - KH + 1, W - KW + 1)
    w1e = np.zeros((HID, H, W), np.float64)
    for a in range(KH):
        for b in range(KW):
            w1e[:, a : a + H - KH + 1, b : b + W - KW + 1] += w1_r * cw[a, b]
    return w1e.reshape(HID, CIN).astype(np.float32)


def _core_x(x_shard):
    """Pre-tile one core's x rows [BS, 784] into the device layout:
    xb [nload, k, c, n] (features 0..767) and xtl [16, BS] (tail)."""
    xb = np.ascontiguousarray(
        x_shard[:, : KC * KCH]
        .reshape(BS // NLOAD, NLOAD, KC, KCH)
        .transpose(0, 3, 2, 1)
        .astype(HOST_DT)
    )
    xtl = np.ascontiguousarray(x_shard[:, KC * KCH :].T.astype(HOST_DT))
    return xb, xtl


def _run(x, conv_w, w1, b1, w2, b2, trace=False):
    x = np.asarray(x, np.float32)
    w1e_t = np.ascontiguousarray(_fuse_conv_fc1(conv_w, w1).T.astype(HOST_DT))
    w2t = np.ascontiguousarray(np.asarray(w2, np.float32).T.astype(HOST_DT))
    b1c = np.ascontiguousarray(np.asarray(b1, np.float32).reshape(HID, 1))
    b2c = np.ascontiguousarray(np.asarray(b2, np.float32).reshape(OUT, 1))

    nc = _build_nc()
    in_maps = []
    for c in range(NCORES):
        xb, xtl = _core_x(x[c * BS : (c + 1) * BS])
        in_maps.append(
            {"xb": xb, "xtl": xtl, "w1t": w1e_t, "b1d": b1c, "w2t": w2t, "b2d": b2c}
        )
    res = run_bass_kernel_spmd(nc, in_maps, list(range(NCORES)), trace=trace)

    y = np.empty((B_TOTAL, OUT), np.float32)
    for c, r in enumerate(res.results):
        y[c * BS : (c + 1) * BS] = r["yt"].T
    return y, res


def kernel(x, conv_w, w1, b1, w2, b2):
    y, _ = _run(x, conv_w, w1, b1, w2, b2)
    return y


# revision 58
# speedup vs baseline: 1.0278x; 1.0278x over previous
"""Trainium2 Bass kernel for nn_DigitConvolutionalModel (dense_cnn).

Model: y = relu(conv3x3(x) @ w1.T + b1) @ w2.T + b2, x: [65536, 784] f32.

Strategy:
  * The 3x3 valid conv (784 -> 676) and FC1 (676 -> 128) are both linear,
    so they fuse on the host into one effective weight W1e = w1 @ C with
    shape [128, 784] (C is the sparse conv operator). The device then runs
    a pure GEMM pipeline: y = relu(x @ W1e.T + b1) @ w2.T + b2.
  * Pure data parallel over 8 NeuronCores: each core gets 8192 rows of x.
    No collectives; each core produces its own output shard.
  * Matmul operands travel as fp16 (see MM_MODE below): tf32-class
    accuracy for this model's value ranges, 1 cycle/row on the PE, and
    half the HBM traffic for x. All accumulation stays fp32 in PSUM.
  * Per-core x shards are pre-tiled on the host into the exact device
    layout [load, partition, chunk, col] so every 1.6 MB x DMA is one
    fully contiguous region (128 descriptors x 12 KB) — the contraction
    dim lands on SBUF partitions with no on-chip transposes (DMA x-bar
    transpose wouldn't help: 2-byte dtype only, and strided fallbacks are
    descriptor-bound). The K=784 contraction is split 6x128 + 16 so the
    bulk loads use all 128 partitions / 16 SDMA engines; the 16-row tail
    for the whole batch is one upfront 256 KB load.
  * Per 1024-column load: two PSUM banks of 7 accumulating FC1 matmuls
    each (si-outer so bank 0's relu overlaps bank 1's matmuls), fused
    bias+ReLU on the vector engine (PSUM -> SBUF fp16), one [10, 512]
    FC2 matmul per bank, FC2 bias on the scalar engine, SWDGE store.
    Output returns as yT [10, 8192] per core; the host transposes/concats.
  * Tile/walrus quirks handled explicitly: this walrus allows ONE sync
    wait per instruction, so multi-waits are split into event-semaphore
    chains (bass_rust.generate_event_semaphores) and tiny dummy bf16
    ldweights "probes" absorb cross-engine waits into the PE stream ahead
    of each matmul group. A dozen dummy matmuls during the DMA-bound
    startup window pre-warm the PE's HAM clock gate to 2.4 GHz.
"""

import os

import numpy as np

import concourse.bass as bass
import concourse.mybir as mybir
import concourse.tile as tile
from concourse.bass import ts
from concourse.bass_utils import run_bass_kernel_spmd

H = W = 28
KH = KW = 3
CIN = H * W  # 784
HID = 128
OUT = 10
B_TOTAL = 65536
NCORES = 8
BS = B_TOTAL // NCORES  # 8192 rows per core
NB = 512  # batch columns per psum block (fp32 PSUM bank limit)
NBLK = BS // NB  # 16
NLOAD = 1024  # batch columns per x DMA (~3.2 MB transfers)
NSUB = NLOAD // NB  # psum blocks per load
# contraction split: 6 full-partition chunks of 128 (keeps all 16 SDMA
# engines loaded on the big x DMAs) + a 16-row tail chunk
KCH = 128
KC = 6  # full chunks (6 * 128 = 768)
KTAIL = CIN - KC * KCH  # 16

# Matmul operand dtype. fp16 (e5m10): tf32-class accuracy for this model's
# value ranges (|x|<6, |h|<13), 1 cycle/row on the PE with fast weight
# load, and half the HBM bytes for x. "f32r" = single-pass reduced fp32
# (same accuracy class, but 4-byte DMA traffic); "f32" = exact.
MM_MODE = os.environ.get("BASS_MM_DT", "f16")
if os.environ.get("BASS_FP32R") == "0":  # legacy switch used by simcheck
    MM_MODE = "f32"
HOST_DT = np.float16 if MM_MODE == "f16" else np.float32


def _build_nc():
    f32 = mybir.dt.float32
    # Matmul-operand dtype. (For f32r, the BIR verifier requires fp32r
    # matmul inputs to be *typed* fp32r at their producer, so the DRAM
    # tensors and SBUF tiles feeding matmuls carry this dtype.)
    mdt = {
        "f16": mybir.dt.float16,
        "f32r": mybir.dt.float32r,
        "f32": f32,
    }[MM_MODE]
    nc = bass.Bass()
    # x big part, host-pretiled to [load, partition, chunk, col] so each
    # per-load DMA is one contiguous region (128 descriptors x 24 KB)
    xb = nc.dram_tensor(
        "xb", [BS // NLOAD, KCH, KC, NLOAD], mdt, kind="ExternalInput"
    )
    # x contraction tail (features 768..783) for the whole batch
    xtl = nc.dram_tensor("xtl", [KTAIL, BS], mdt, kind="ExternalInput")
    w1t = nc.dram_tensor("w1t", [CIN, HID], mdt, kind="ExternalInput")
    b1d = nc.dram_tensor("b1d", [HID, 1], f32, kind="ExternalInput")
    w2t = nc.dram_tensor("w2t", [HID, OUT], mdt, kind="ExternalInput")
    b2d = nc.dram_tensor("b2d", [OUT, 1], f32, kind="ExternalInput")
    yt = nc.dram_tensor("yt", [OUT, BS], f32, kind="ExternalOutput")

    with tile.TileContext(nc) as tc:
        with (
            tc.tile_pool(name="consts", bufs=1) as consts,
            # buffer depths sized for the 2-byte path; the 4-byte fallback
            # modes halve them to stay inside SBUF
            tc.tile_pool(name="xin", bufs=6 if MM_MODE == "f16" else 3) as xin,
            tc.tile_pool(name="hpool", bufs=16 if MM_MODE == "f16" else 8) as hpool,
            tc.tile_pool(name="opool", bufs=8) as opool,
            tc.tile_pool(name="ps1", bufs=4, space="PSUM") as ps1p,
            tc.tile_pool(name="ps2", bufs=2, space="PSUM") as ps2p,
        ):
            # Tapered block schedule: 512-column blocks at the start (so the
            # first FC1 only waits on a quarter-size transfer) and at the
            # end (shorter pipeline drain); full 1024-column blocks between.
            sched = (
                [(0, NB), (NB, NB)]
                + [(cs, NLOAD) for cs in range(NLOAD, BS - NLOAD, NLOAD)]
                + [(BS - NLOAD, NB), (BS - NB, NB)]
            )

            def x_src(colstart, ncols):
                li, off = divmod(colstart, NLOAD)
                return xb[li][:, :, off : off + ncols]

            # Issue the first x block before anything else so the main DMA
            # stream starts as early as possible (weights are tiny and only
            # gate the PE, which has plenty of slack).
            x_first = xin.tile([KCH, KC, sched[0][1]], mdt, tag="x_s")
            nc.sync.dma_start(x_first[:], x_src(*sched[0]))

            # FC1 weight, chunked [k, chunk, hid]: partition k in 0..127,
            # chunk c selects rows c*128..c*128+127 of w1t; plus 16-row tail.
            w1_t = consts.tile([KCH, KC, HID], mdt)
            nc.sync.dma_start(
                w1_t[:], w1t[0 : KC * KCH, :].rearrange("(c k) m -> k c m", k=KCH)
            )
            w1_tail = consts.tile([KTAIL, HID], mdt)
            nc.sync.dma_start(w1_tail[:], w1t[KC * KCH :, :])
            b1_t = consts.tile([HID, 1], f32)
            nc.sync.dma_start(b1_t[:], b1d[:])
            w2_t = consts.tile([HID, OUT], mdt)
            nc.sync.dma_start(w2_t[:], w2t[:])
            b2_t = consts.tile([OUT, 1], f32)
            nc.sync.dma_start(b2_t[:], b2d[:])

            # Pre-touch the bias tiles on their consumer engines (b1 on DVE,
            # b2 on ACT) so the relu / bias-add instructions don't need a
            # second sync-wait for the bias DMA (walrus: 1 wait per inst).
            b1_probe = consts.tile([1, 1], f32)
            nc.vector.tensor_copy(b1_probe[:], b1_t[0:1, 0:1])
            b2_probe = consts.tile([1, 1], f32)
            nc.scalar.copy(b2_probe[:], b2_t[0:1, 0:1])

            # Matmuls self-load their weights, so every semaphore wait lands
            # on the Matmult itself — and walrus only allows one sync-wait
            # there. Tiny dummy bf16 ldweights "probes" reading 1 element of
            # a tile absorb the cross-engine waits into the PE's in-order
            # stream before each matmul group. The loaded garbage weight is
            # irrelevant (the real matmuls self-load).
            def probe(ap):
                nc.tensor.ldweights(ap[0:1, 0:1].bitcast(mybir.dt.bfloat16))

            # the 16-row contraction tail for the whole batch, loaded once
            x_tl = consts.tile([KTAIL, BS], mdt)
            nc.sync.dma_start(x_tl[:], xtl[:])

            probe(w1_t[:, 0, :])
            probe(w1_tail[:])
            probe(x_tl[:])
            probe(w2_t[:])

            # HAM warm-up: the PE clock gate defaults to 1.2 GHz and only
            # ramps to 2.4 GHz after ~3.4us of sustained activity. The PE is
            # idle anyway until the first x load lands (~13us), so burn that
            # window on dummy matmuls over a zeroed scratch tile; the real
            # matmuls then start at full clock.
            scratch = consts.tile([KCH, NB], mdt)
            nc.gpsimd.memset(scratch[:], 0.0)
            psd = ps1p.tile([HID, NB], f32, tag="ps")
            for _ in range(4):
                nc.tensor.matmul(
                    psd[:], scratch[:, 0:HID], scratch[:], start=True, stop=True
                )

            # Per block: FC1 runs si-outer (bank si finishes early so its
            # relu overlaps the other bank's matmuls), then per-bank
            # relu -> FC2, then one bias-add + store for the whole block.
            for bi, (colstart, ncols) in enumerate(sched):
                nsub = ncols // NB
                if bi == 0:
                    x_t = x_first
                else:
                    tag = "x_t" if ncols == NLOAD else "x_s"
                    x_t = xin.tile([KCH, KC, ncols], mdt, tag=tag)
                    nc.sync.dma_start(x_t[:], x_src(colstart, ncols))

                probe(x_t[:, 0, :])
                # si-outer: bank si finishes its 7-matmul accumulation
                # early, so its relu overlaps the other bank's matmuls
                pss = []
                for si in range(nsub):
                    ps_si = ps1p.tile([HID, NB], f32, tag="ps")
                    for c in range(KC):
                        nc.tensor.matmul(
                            ps_si[:],
                            w1_t[:, c, :],
                            x_t[:, c, ts(si, NB)],
                            start=(c == 0),
                            stop=False,
                        )
                    nc.tensor.matmul(
                        ps_si[:],
                        w1_tail[:],
                        x_tl[:, ts(colstart // NB + si, NB)],
                        start=False,
                        stop=True,
                    )
                    pss.append(ps_si)

                # ps2 is always the 2-bank shape so all blocks share slots
                ps2 = ps2p.tile([OUT, NSUB, NB], f32, tag="ps2")
                for si in range(nsub):
                    # relu+bias on DVE: h = max(ps + b1, 0)
                    h = hpool.tile([HID, NB], mdt, tag="h")
                    nc.vector.tensor_scalar(
                        h[:],
                        pss[si][:],
                        b1_t[:],
                        0.0,
                        mybir.AluOpType.add,
                        mybir.AluOpType.max,
                    )
                    probe(h[:])
                    nc.tensor.matmul(
                        ps2[:, si, :], w2_t[:], h[:], start=True, stop=True
                    )

                # FC2 bias on the (otherwise idle) scalar engine, store via
                # SWDGE so the ACT sequencer isn't serialized behind it
                o = opool.tile([OUT, nsub, NB], f32, tag="o")
                nc.scalar.activation(
                    o[:],
                    ps2[:, 0:nsub, :],
                    mybir.ActivationFunctionType.Identity,
                    bias=b2_t[:],
                )
                nc.gpsimd.dma_start(yt[:, colstart : colstart + ncols], o[:])

    # This walrus build allows one sync-wait per instruction; Tile emits
    # multi-waits (e.g. slot-recycle WAW + readers-release on DMAs). Split
    # them into event-semaphore chains, same as bacc.compile() does.
    import bass_rust

    bass_rust.generate_event_semaphores(nc)
    return nc


def _fuse_conv_fc1(conv_w, w1):
    """W1e = w1 @ C where C is the 3x3 valid-conv operator [676, 784]."""
    cw = np.asarray(conv_w, np.float64).reshape(KH, KW)
    w1_r = np.asarray(w1, np.float64).reshape(HID, H - KH + 1, W - KW + 1)
    w1e = np.zeros((HID, H, W), np.float64)
    for a in range(KH):
        for b in range(KW):
            w1e[:, a : a + H - KH + 1, b : b + W - KW + 1] += w1_r * cw[a, b]
    return w1e.reshape(HID, CIN).astype(np.float32)


def _core_x(x_shard):
    """Pre-tile one core's x rows [BS, 784] into the device layout:
    xb [nload, k, c, n] (features 0..767) and xtl [16, BS] (tail)."""
    xb = np.ascontiguousarray(
        x_shard[:, : KC * KCH]
        .reshape(BS // NLOAD, NLOAD, KC, KCH)
        .transpose(0, 3, 2, 1)
        .astype(HOST_DT)
    )
    xtl = np.ascontiguousarray(x_shard[:, KC * KCH :].T.astype(HOST_DT))
    return xb, xtl


def _run(x, conv_w, w1, b1, w2, b2, trace=False):
    x = np.asarray(x, np.float32)
    w1e_t = np.ascontiguousarray(_fuse_conv_fc1(conv_w, w1).T.astype(HOST_DT))
    w2t = np.ascontiguousarray(np.asarray(w2, np.float32).T.astype(HOST_DT))
    b1c = np.ascontiguousarray(np.asarray(b1, np.float32).reshape(HID, 1))
    b2c = np.ascontiguousarray(np.asarray(b2, np.float32).reshape(OUT, 1))

    nc = _build_nc()
    in_maps = []
    for c in range(NCORES):
        xb, xtl = _core_x(x[c * BS : (c + 1) * BS])
        in_maps.append(
            {"xb": xb, "xtl": xtl, "w1t": w1e_t, "b1d": b1c, "w2t": w2t, "b2d": b2c}
        )
    res = run_bass_kernel_spmd(nc, in_maps, list(range(NCORES)), trace=trace)

    y = np.empty((B_TOTAL, OUT), np.float32)
    for c, r in enumerate(res.results):
        y[c * BS : (c + 1) * BS] = r["yt"].T
    return y, res


def kernel(x, conv_w, w1, b1, w2, b2):
    y, _ = _run(x, conv_w, w1, b1, w2, b2)
    return y


# revision 59
# speedup vs baseline: 1.0526x; 1.0241x over previous
"""Trainium2 Bass kernel for nn_DigitConvolutionalModel (dense_cnn).

Model: y = relu(conv3x3(x) @ w1.T + b1) @ w2.T + b2, x: [65536, 784] f32.

Strategy:
  * The 3x3 valid conv (784 -> 676) and FC1 (676 -> 128) are both linear,
    so they fuse on the host into one effective weight W1e = w1 @ C with
    shape [128, 784] (C is the sparse conv operator). The device then runs
    a pure GEMM pipeline: y = relu(x @ W1e.T + b1) @ w2.T + b2.
  * Pure data parallel over 8 NeuronCores: each core gets 8192 rows of x.
    No collectives; each core produces its own output shard.
  * Matmul operands travel as fp16 (see MM_MODE below): tf32-class
    accuracy for this model's value ranges, 1 cycle/row on the PE, and
    half the HBM traffic for x. All accumulation stays fp32 in PSUM.
  * Per-core x shards are pre-tiled on the host into the exact device
    layout [load, partition, chunk, col] so every 1.6 MB x DMA is one
    fully contiguous region (128 descriptors x 12 KB) — the contraction
    dim lands on SBUF partitions with no on-chip transposes (DMA x-bar
    transpose wouldn't help: 2-byte dtype only, and strided fallbacks are
    descriptor-bound). The K=784 contraction is split 6x128 + 16 so the
    bulk loads use all 128 partitions / 16 SDMA engines; the 16-row tail
    for the whole batch is one upfront 256 KB load.
  * Per 1024-column load: two PSUM banks of 7 accumulating FC1 matmuls
    each (si-outer so bank 0's relu overlaps bank 1's matmuls), fused
    bias+ReLU on the vector engine (PSUM -> SBUF fp16), one [10, 512]
    FC2 matmul per bank, FC2 bias on the scalar engine, SWDGE store.
    Output returns as yT [10, 8192] per core; the host transposes/concats.
  * Tile/walrus quirks handled explicitly: this walrus allows ONE sync
    wait per instruction, so multi-waits are split into event-semaphore
    chains (bass_rust.generate_event_semaphores) and tiny dummy bf16
    ldweights "probes" absorb cross-engine waits into the PE stream ahead
    of each matmul group. A dozen dummy matmuls during the DMA-bound
    startup window pre-warm the PE's HAM clock gate to 2.4 GHz.
"""

import os

import numpy as np

import concourse.bass as bass
import concourse.mybir as mybir
import concourse.tile as tile
from concourse.bass import ts
from concourse.bass_utils import run_bass_kernel_spmd

H = W = 28
KH = KW = 3
CIN = H * W  # 784
HID = 128
OUT = 10
B_TOTAL = 65536
NCORES = 8
BS = B_TOTAL // NCORES  # 8192 rows per core
NB = 512  # batch columns per psum block (fp32 PSUM bank limit)
NBLK = BS // NB  # 16
NLOAD = 1024  # batch columns per x DMA (~3.2 MB transfers)
NSUB = NLOAD // NB  # psum blocks per load
# contraction split: 6 full-partition chunks of 128 (keeps all 16 SDMA
# engines loaded on the big x DMAs) + a 16-row tail chunk
KCH = 128
KC = 6  # full chunks (6 * 128 = 768)
KTAIL = CIN - KC * KCH  # 16

# Matmul operand dtype. fp16 (e5m10): tf32-class accuracy for this model's
# value ranges (|x|<6, |h|<13), 1 cycle/row on the PE with fast weight
# load, and half the HBM bytes for x. "f32r" = single-pass reduced fp32
# (same accuracy class, but 4-byte DMA traffic); "f32" = exact.
MM_MODE = os.environ.get("BASS_MM_DT", "f16")
if os.environ.get("BASS_FP32R") == "0":  # legacy switch used by simcheck
    MM_MODE = "f32"
HOST_DT = np.float16 if MM_MODE == "f16" else np.float32


def _build_nc():
    f32 = mybir.dt.float32
    # Matmul-operand dtype. (For f32r, the BIR verifier requires fp32r
    # matmul inputs to be *typed* fp32r at their producer, so the DRAM
    # tensors and SBUF tiles feeding matmuls carry this dtype.)
    mdt = {
        "f16": mybir.dt.float16,
        "f32r": mybir.dt.float32r,
        "f32": f32,
    }[MM_MODE]
    nc = bass.Bass()
    # x big part, host-pretiled to [load, partition, chunk, col] so each
    # per-load DMA is one contiguous region (128 descriptors x 24 KB)
    xb = nc.dram_tensor(
        "xb", [BS // NLOAD, KCH, KC, NLOAD], mdt, kind="ExternalInput"
    )
    # x contraction tail (features 768..783) for the whole batch
    xtl = nc.dram_tensor("xtl", [KTAIL, BS], mdt, kind="ExternalInput")
    w1t = nc.dram_tensor("w1t", [CIN, HID], mdt, kind="ExternalInput")
    b1d = nc.dram_tensor("b1d", [HID, 1], f32, kind="ExternalInput")
    w2t = nc.dram_tensor("w2t", [HID, OUT], mdt, kind="ExternalInput")
    b2d = nc.dram_tensor("b2d", [OUT, 1], f32, kind="ExternalInput")
    yt = nc.dram_tensor("yt", [OUT, BS], f32, kind="ExternalOutput")

    with tile.TileContext(nc) as tc:
        with (
            tc.tile_pool(name="consts", bufs=1) as consts,
            # buffer depths sized for the 2-byte path; the 4-byte fallback
            # modes halve them to stay inside SBUF
            tc.tile_pool(name="xin", bufs=6 if MM_MODE == "f16" else 3) as xin,
            tc.tile_pool(name="hpool", bufs=16 if MM_MODE == "f16" else 8) as hpool,
            tc.tile_pool(name="opool", bufs=8) as opool,
            tc.tile_pool(name="ps1", bufs=4, space="PSUM") as ps1p,
            tc.tile_pool(name="ps2", bufs=2, space="PSUM") as ps2p,
        ):
            # Tapered block schedule: 512-column blocks at the start (so the
            # first FC1 only waits on a quarter-size transfer) and at the
            # end (shorter pipeline drain); full 1024-column blocks between.
            sched = (
                [(0, NB), (NB, NB)]
                + [(cs, NLOAD) for cs in range(NLOAD, BS - NLOAD, NLOAD)]
                + [(BS - NLOAD, NB), (BS - NB, NB)]
            )

            def x_src(colstart, ncols):
                li, off = divmod(colstart, NLOAD)
                return xb[li][:, :, off : off + ncols]

            # Issue the first x block before anything else so the main DMA
            # stream starts as early as possible (weights are tiny and only
            # gate the PE, which has plenty of slack).
            x_first = xin.tile([KCH, KC, sched[0][1]], mdt, tag="x_s")
            nc.sync.dma_start(x_first[:], x_src(*sched[0]))

            # FC1 weight, chunked [k, chunk, hid]: partition k in 0..127,
            # chunk c selects rows c*128..c*128+127 of w1t; plus 16-row tail.
            w1_t = consts.tile([KCH, KC, HID], mdt)
            nc.sync.dma_start(
                w1_t[:], w1t[0 : KC * KCH, :].rearrange("(c k) m -> k c m", k=KCH)
            )
            w1_tail = consts.tile([KTAIL, HID], mdt)
            nc.sync.dma_start(w1_tail[:], w1t[KC * KCH :, :])
            b1_t = consts.tile([HID, 1], f32)
            nc.sync.dma_start(b1_t[:], b1d[:])
            w2_t = consts.tile([HID, OUT], mdt)
            nc.sync.dma_start(w2_t[:], w2t[:])
            b2_t = consts.tile([OUT, 1], f32)
            nc.sync.dma_start(b2_t[:], b2d[:])

            # Pre-touch the bias tiles on their consumer engines (b1 on DVE,
            # b2 on ACT) so the relu / bias-add instructions don't need a
            # second sync-wait for the bias DMA (walrus: 1 wait per inst).
            b1_probe = consts.tile([1, 1], f32)
            nc.vector.tensor_copy(b1_probe[:], b1_t[0:1, 0:1])
            b2_probe = consts.tile([1, 1], f32)
            nc.scalar.copy(b2_probe[:], b2_t[0:1, 0:1])

            # Matmuls self-load their weights, so every semaphore wait lands
            # on the Matmult itself — and walrus only allows one sync-wait
            # there. Tiny dummy bf16 ldweights "probes" reading 1 element of
            # a tile absorb the cross-engine waits into the PE's in-order
            # stream before each matmul group. The loaded garbage weight is
            # irrelevant (the real matmuls self-load).
            def probe(ap):
                nc.tensor.ldweights(ap[0:1, 0:1].bitcast(mybir.dt.bfloat16))

            # the 16-row contraction tail for the whole batch, loaded once
            x_tl = consts.tile([KTAIL, BS], mdt)
            nc.sync.dma_start(x_tl[:], xtl[:])

            probe(w1_t[:, 0, :])
            probe(w1_tail[:])
            probe(x_tl[:])
            probe(w2_t[:])

            # HAM warm-up: the PE clock gate defaults to 1.2 GHz and only
            # ramps to 2.4 GHz after ~3.4us of sustained activity. The PE is
            # idle anyway until the first x load lands (~13us), so burn that
            # window on dummy matmuls over a zeroed scratch tile; the real
            # matmuls then start at full clock.
            scratch = consts.tile([KCH, NB], mdt)
            nc.gpsimd.memset(scratch[:], 0.0)
            psd = ps1p.tile([HID, NB], f32, tag="ps")
            for _ in range(4):
                nc.tensor.matmul(
                    psd[:], scratch[:, 0:HID], scratch[:], start=True, stop=True
                )

            # Per block: FC1 runs si-outer (bank si finishes early so its
            # relu overlaps the other bank's matmuls), then per-bank
            # relu -> FC2, then one bias-add + store for the whole block.
            for bi, (colstart, ncols) in enumerate(sched):
                nsub = ncols // NB
                if bi == 0:
                    x_t = x_first
                else:
                    tag = "x_t" if ncols == NLOAD else "x_s"
                    x_t = xin.tile([KCH, KC, ncols], mdt, tag=tag)
                    nc.sync.dma_start(x_t[:], x_src(colstart, ncols))

                probe(x_t[:, 0, :])
                # si-outer: bank si finishes its 7-matmul accumulation
                # early, so its relu overlaps the other bank's matmuls
                pss = []
                for si in range(nsub):
                    ps_si = ps1p.tile([HID, NB], f32, tag="ps")
                    for c in range(KC):
                        nc.tensor.matmul(
                            ps_si[:],
                            w1_t[:, c, :],
                            x_t[:, c, ts(si, NB)],
                            start=(c == 0),
                            stop=False,
                        )
                    nc.tensor.matmul(
                        ps_si[:],
                        w1_tail[:],
                        x_tl[:, ts(colstart // NB + si, NB)],
                        start=False,
                        stop=True,
                    )
                    pss.append(ps_si)

                # ps2 is always the 2-bank shape so all blocks share slots
                ps2 = ps2p.tile([OUT, NSUB, NB], f32, tag="ps2")
                for si in range(nsub):
                    # relu+bias on DVE: h = max(ps + b1, 0)
                    h = hpool.tile([HID, NB], mdt, tag="h")
                    nc.vector.tensor_scalar(
                        h[:],
                        pss[si][:],
                        b1_t[:],
                        0.0,
                        mybir.AluOpType.add,
                        mybir.AluOpType.max,
                    )
                    probe(h[:])
                    nc.tensor.matmul(
                        ps2[:, si, :], w2_t[:], h[:], start=True, stop=True
                    )

                # FC2 bias on the (otherwise idle) scalar engine, store via
                # SWDGE so the ACT sequencer isn't serialized behind it
                o = opool.tile([OUT, nsub, NB], f32, tag="o")
                nc.scalar.activation(
                    o[:],
                    ps2[:, 0:nsub, :],
                    mybir.ActivationFunctionType.Identity,
                    bias=b2_t[:],
                )
                nc.scalar.dma_start(yt[:, colstart : colstart + ncols], o[:])

    # This walrus build allows one sync-wait per instruction; Tile emits
    # multi-waits (e.g. slot-recycle WAW + readers-release on DMAs). Split
    # them into event-semaphore chains, same as bacc.compile() does.
    import bass_rust

    bass_rust.generate_event_semaphores(nc)
    return nc


def _fuse_conv_fc1(conv_w, w1):
    """W1e = w1 @ C where C is the 3x3 valid-conv operator [676, 784]."""
    cw = np.asarray(conv_w, np.float64).reshape(KH, KW)
    w1_r = np.asarray(w1, np.float64).reshape(HID, H - KH + 1, W - KW + 1)
    w1e = np.zeros((HID, H, W), np.float64)
    for a in range(KH):
        for b in range(KW):
            w1e[:, a : a + H - KH + 1, b : b + W - KW + 1] += w1_r * cw[a, b]
    return w1e.reshape(HID, CIN).astype(np.float32)


def _core_x(x_shard):
    """Pre-tile one core's x rows [BS, 784] into the device layout:
    xb [nload, k, c, n] (features 0..767) and xtl [16, BS] (tail)."""
    xb = np.ascontiguousarray(
        x_shard[:, : KC * KCH]
        .reshape(BS // NLOAD, NLOAD, KC, KCH)
        .transpose(0, 3, 2, 1)
        .astype(HOST_DT)
    )
    xtl = np.ascontiguousarray(x_shard[:, KC * KCH :].T.astype(HOST_DT))
    return xb, xtl


def _run(x, conv_w, w1, b1, w2, b2, trace=False):
    x = np.asarray(x, np.float32)
    w1e_t = np.ascontiguousarray(_fuse_conv_fc1(conv_w, w1).T.astype(HOST_DT))
    w2t = np.ascontiguousarray(np.asarray(w2, np.float32).T.astype(HOST_DT))
    b1c = np.ascontiguousarray(np.asarray(b1, np.float32).reshape(HID, 1))
    b2c = np.ascontiguousarray(np.asarray(b2, np.float32).reshape(OUT, 1))

    nc = _build_nc()
    in_maps = []
    for c in range(NCORES):
        xb, xtl = _core_x(x[c * BS : (c + 1) * BS])
        in_maps.append(
            {"xb": xb, "xtl": xtl, "w1t": w1e_t, "b1d": b1c, "w2t": w2t, "b2d": b2c}
        )
    res = run_bass_kernel_spmd(nc, in_maps, list(range(NCORES)), trace=trace)

    y = np.empty((B_TOTAL, OUT), np.float32)
    for c, r in enumerate(res.results):
        y[c * BS : (c + 1) * BS] = r["yt"].T
    return y, res


def kernel(x, conv_w, w1, b1, w2, b2):
    y, _ = _run(x, conv_w, w1, b1, w2, b2)
    return y
